# revision 44
# baseline (speedup 1.0000x reference)
"""Trainium2 Bass kernel for nn_AttentionBlock (GroupNorm + 2-head attention + proj + residual).

Full inputs: x (16, 256, 32, 32) f32, gn_w/gn_b (256,), wq/wk/wv/wp (256, 256).
Sharding: pure data-parallel over batch — 16 / 8 cores = 2 batch elements per core.
No collectives; outputs concatenated on host.

Per-core dataflow (per batch element, channels-on-partitions):
  x (256, 1024)  -> GroupNorm(4 groups) via free-dim reduces + PE group-mask matmuls
  xn -> q, k (256, 1024) = Wq/Wk @ xn ;  vT (1024, 256) = xn^T @ Wv^T
  per head h (hd = 128): ST_jt (j, i) = k_h[:, jt]^T q_h  (scores transposed)
                         ET = exp(scale * ST)      (ACT, fused scale)
                         U (c, i)  = sum_jt vT_jt_h^T @ ET_jt   (PSUM accum)
                         D (., i)  = sum_jt ones^T @ ET_jt      (softmax denom, replicated)
                         ao_h = U * (1/D)
  out = Wp @ [ao_0; ao_1] + x
All big matmuls run in bf16 (fp32 PSUM accumulation); GroupNorm stats/chain in
fp32. A bf16 copy of x feeds the GN/xn path so the critical input DMA is half
the bytes; fp32 x is only used for the residual. Dummy bf16 warmup matmuls trip
the PE HAM clock gate to 2.4 GHz before real work arrives. Weights and GN
constants arrive as two const-blob DMAs. Emission order interleaves the two
batch elements so b1's GN/QKV hide under b0's attention.
"""

import numpy as np

import concourse.bass as bass
import concourse.tile as tile
from concourse import bacc, mybir
from concourse.bass_utils import run_bass_kernel_spmd

N_CORES = 8
B = 16
BPC = B // N_CORES  # batch elements per core
C = 256
H = W = 32
N = H * W  # 1024 spatial positions
HEADS = 2
HD = C // HEADS  # 128 head dim
G = 4  # groupnorm groups
GSIZE = C // G  # 64 channels per group
EPS = 1e-5
ATT_SCALE = float((C * HEADS) ** -0.5)
P = 128  # partitions
CT = C // P  # channel tiles (2)
FT = 512  # matmul moving-dim tile (one fp32 PSUM bank)
NT = N // FT  # n tiles per matmul row pass (2)
JT = N // P  # j tiles (8)
NG = GSIZE * N  # elements per (batch, group)

# const blob column offsets; region [0, CB_W) is the bf16 weight blob,
# [CB_W, CB_F) the fp32 GN-const blob.
OFF_W = 0  # 4 weights (q,k,v,p), each CT*C = 512 cols
OFF_ONES = 2048  # 128 cols of 1.0
CB_W = 2176
OFF_GNWB = 2176  # per ct: 2 cols
OFF_GMASK = 2184  # per ct: G cols
OFF_GMT = 2192  # per ct: 128 cols (values live in rows 0..G-1)
OFF_EPS = 2448  # one col: EPS in rows 0..G-1
CB_F = 2452

f32 = mybir.dt.float32
f32r = mybir.dt.float32r
bf16 = mybir.dt.bfloat16
MM_DT = bf16  # dtype of all big-matmul operands
N_WARMUP = 16  # PE warmup matmuls (~5us busy) to trip the HAM clock gate early
AF = mybir.ActivationFunctionType
ALU = mybir.AluOpType
AX = mybir.AxisListType


def build_bass(bpc=BPC):
    nc = bacc.Bacc("TRN2", target_bir_lowering=False, debug=False)

    x_d = nc.dram_tensor("x", [bpc, C, N], f32, kind="ExternalInput").ap()
    xb_d = nc.dram_tensor("xb", [bpc, C, N], bf16, kind="ExternalInput").ap()
    cbw_d = nc.dram_tensor("cbw", [P, CB_W], MM_DT, kind="ExternalInput").ap()
    cbg_d = nc.dram_tensor("cbg", [P, CB_F - CB_W], f32, kind="ExternalInput").ap()
    out_d = nc.dram_tensor("out", [bpc, C, N], f32, kind="ExternalOutput").ap()

    with tile.TileContext(nc) as tc:
        with (
            tc.tile_pool(name="consts", bufs=1) as consts,
            tc.tile_pool(name="xp", bufs=2) as xp,
            tc.tile_pool(name="xnp", bufs=2) as xnp,
            tc.tile_pool(name="qk", bufs=2) as qk,
            tc.tile_pool(name="vp", bufs=2) as vp,
            tc.tile_pool(name="etp", bufs=2) as etp,
            tc.tile_pool(name="sm", bufs=2) as sm,
            tc.tile_pool(name="scr", bufs=2) as scr,
            tc.tile_pool(name="aop", bufs=2) as aop,
            tc.tile_pool(name="op", bufs=2) as op,
            tc.tile_pool(name="pmm", bufs=2, space="PSUM") as pmm,
            tc.tile_pool(name="pacc", bufs=1, space="PSUM") as pacc,
        ):
            # ---- PE warmup: dense dummy matmuls (no input deps) so the HAM
            # clock gate reaches K=8/8 before the real matmuls start.
            wtile = consts.tile([P, FT], bf16, tag="warm")
            nc.gpsimd.memset(wtile[:], 0.0)
            wps = pacc.tile([P, FT], f32, tag="u")
            for _ in range(N_WARMUP):
                nc.tensor.matmul(
                    wps[:], wtile[:, 0:P], wtile[:], start=True, stop=True
                )

            # ---- inputs: GN consts + x tiles spread over several DMA queues
            # (sync/gpsimd/vector issue to different HWDGE queues -> parallel)
            cbg = consts.tile([P, CB_F - CB_W], f32, tag="cbg")
            nc.sync.dma_start(cbg[:], cbg_d[:])
            # bf16 copy of x feeds GN stats + xn (half the critical bytes);
            # fp32 x arrives later, used only for the residual add.
            dma_engs = [nc.sync, nc.gpsimd, nc.scalar]
            xb_all = []
            for b in range(bpc):
                xbs = []
                for ct in range(CT):
                    xt = xp.tile([P, N], bf16, tag=f"xb{ct}")
                    eng = dma_engs[(b * CT + ct) % 3]
                    eng.dma_start(xt[:], xb_d[b, ct * P : (ct + 1) * P, :])
                    xbs.append(xt)
                xb_all.append(xbs)
            xs_all = []
            for b in range(bpc):
                xs = []
                for ct in range(CT):
                    xt = xp.tile([P, N], f32, tag=f"x{ct}")
                    eng = dma_engs[(b * CT + ct) % 3]
                    eng.dma_start(xt[:], x_d[b, ct * P : (ct + 1) * P, :])
                    xs.append(xt)
                xs_all.append(xs)

            cbw = consts.tile([P, CB_W], MM_DT, tag="cbw")
            nc.scalar.dma_start(cbw[:], cbw_d[:])

            def w_ap(i, kt):  # (128, C) lhsT/rhs slice of weight i, k-tile kt
                base = OFF_W + i * (CT * C) + kt * C
                return cbw[:, base : base + C]

            def g_ap(off):
                return off - CB_W

            gw = [
                cbg[:, g_ap(OFF_GNWB) + ct * 2 : g_ap(OFF_GNWB) + (ct + 1) * 2]
                for ct in range(CT)
            ]
            gm = [
                cbg[:, g_ap(OFF_GMASK) + ct * G : g_ap(OFF_GMASK) + (ct + 1) * G]
                for ct in range(CT)
            ]
            gmt = [
                cbg[0:G, g_ap(OFF_GMT) + ct * P : g_ap(OFF_GMT) + (ct + 1) * P]
                for ct in range(CT)
            ]
            ones = cbw[:, OFF_ONES : OFF_ONES + P]
            eps_ap = cbg[0:G, g_ap(OFF_EPS) : g_ap(OFF_EPS) + 1]
            WQ, WK, WV, WP_ = 0, 1, 2, 3

            s12_all = {}

            def gn_stats(b):
                xbs = xb_all[b]
                s12s = []
                for ct in range(CT):
                    s12 = sm.tile([P, 2], f32, tag=f"s12_{ct}")
                    nc.vector.reduce_sum(s12[:, 0:1], xbs[ct][:], AX.X)
                    sq = scr.tile([P, N], f32, tag="sq")
                    nc.scalar.activation(
                        sq[:], xbs[ct][:], AF.Square, accum_out=s12[:, 1:2]
                    )
                    s12s.append(s12)
                s12_all[b] = s12s

            def gn_chain(b):
                """gstats matmul -> rstd/mean -> per-channel scale/bias -> xn."""
                s12s = s12_all[b]
                xbs = xb_all[b]
                gstats = pmm.tile([G, 2], f32, tag="mm")
                for ct in range(CT):
                    nc.tensor.matmul(
                        gstats[:],
                        gm[ct],
                        s12s[ct][:],
                        start=(ct == 0),
                        stop=(ct == CT - 1),
                    )
                # gstats = [mean, ex2] (masks pre-scaled by 1/NG on host)
                mrs = sm.tile([G, 2], f32, tag="mrs")  # col0 = rstd, col1 = mean
                nc.vector.tensor_copy(mrs[:, 1:2], gstats[:, 0:1])
                negvar = sm.tile([G, 1], f32, tag="negvar")
                nc.vector.scalar_tensor_tensor(
                    negvar[:],
                    mrs[:, 1:2],
                    mrs[:, 1:2],
                    gstats[:, 1:2],
                    ALU.mult,
                    ALU.subtract,
                )
                std = sm.tile([G, 1], f32, tag="std")
                nc.scalar.activation(
                    std[:], negvar[:], AF.Sqrt, bias=eps_ap, scale=-1.0
                )
                nc.vector.reciprocal(mrs[:, 0:1], std[:])

                sbias = []
                for ct in range(CT):
                    bc = pmm.tile([P, 2], f32, tag="mm")
                    nc.tensor.matmul(bc[:], gmt[ct], mrs[:], start=True, stop=True)
                    scale = sm.tile([P, 1], f32, tag=f"scale{ct}")
                    nc.vector.tensor_tensor(scale[:], bc[:, 0:1], gw[ct][:, 0:1], ALU.mult)
                    nbias = sm.tile([P, 1], f32, tag=f"nbias{ct}")
                    nc.vector.tensor_tensor(nbias[:], bc[:, 1:2], scale[:], ALU.mult)
                    nc.vector.tensor_tensor(
                        nbias[:], gw[ct][:, 1:2], nbias[:], ALU.subtract
                    )
                    sbias.append((scale, nbias))

                xns = []
                for ct in range(CT):
                    xn = xnp.tile([P, N], MM_DT, tag=f"xn{ct}")
                    for nt in range(NT):
                        nc.vector.tensor_scalar(
                            xn[:, nt * FT : (nt + 1) * FT],
                            xbs[ct][:, nt * FT : (nt + 1) * FT],
                            sbias[ct][0][:],
                            sbias[ct][1][:],
                            ALU.mult,
                            ALU.add,
                        )
                    xns.append(xn)
                return xns

            def qkv(b, xns):
                qs, ks = [], []
                for wi, outl, name in ((WQ, qs, "q"), (WK, ks, "k")):
                    for ot in range(CT):
                        ps = pmm.tile([P, N], f32, tag="mm")
                        for nt in range(NT):
                            for kt in range(CT):
                                nc.tensor.matmul(
                                    ps[:, nt * FT : (nt + 1) * FT],
                                    w_ap(wi, kt)[:, ot * P : (ot + 1) * P],
                                    xns[kt][:, nt * FT : (nt + 1) * FT],
                                    start=(kt == 0),
                                    stop=(kt == CT - 1),
                                )
                        t = qk.tile([P, N], MM_DT, tag=f"{name}{ot}")
                        for nt in range(NT):
                            sl = slice(nt * FT, (nt + 1) * FT)
                            if name == "q":
                                nc.scalar.copy(t[:, sl], ps[:, sl])
                            else:
                                nc.vector.tensor_copy(t[:, sl], ps[:, sl])
                        outl.append(t)
                vT = vp.tile([P, JT * C], MM_DT, tag="vt")
                for mt in range(JT):
                    ps = pmm.tile([P, C], f32, tag="mm")
                    for kt in range(CT):
                        nc.tensor.matmul(
                            ps[:],
                            xns[kt][:, mt * P : (mt + 1) * P],
                            w_ap(WV, kt),
                            start=(kt == 0),
                            stop=(kt == CT - 1),
                        )
                    nc.vector.tensor_copy(vT[:, mt * C : (mt + 1) * C], ps[:])
                return qs, ks, vT

            def attn(b, qs, ks, vT, filler=None):
                aos = []
                for h in range(HEADS):
                    qh, kh = qs[h], ks[h]
                    et = etp.tile([P, JT * N], MM_DT, tag="et")
                    for jt in range(JT):
                        st = pmm.tile([P, N], f32, tag="mm")
                        for nt in range(NT):
                            nc.tensor.matmul(
                                st[:, nt * FT : (nt + 1) * FT],
                                kh[:, jt * P : (jt + 1) * P],
                                qh[:, nt * FT : (nt + 1) * FT],
                                start=True,
                                stop=True,
                            )
                        nc.scalar.activation(
                            et[:, jt * N : (jt + 1) * N],
                            st[:],
                            AF.Exp,
                            scale=ATT_SCALE,
                        )
                    u = pacc.tile([P, N], f32, tag="u")
                    dd = pacc.tile([P, N], f32, tag="d")
                    for jt in range(JT):
                        if filler is not None and h == HEADS - 1 and jt == JT - 2:
                            filler()
                            filler = None
                        for nt in range(NT):
                            sl = slice(jt * N + nt * FT, jt * N + (nt + 1) * FT)
                            nc.tensor.matmul(
                                u[:, nt * FT : (nt + 1) * FT],
                                vT[:, jt * C + h * HD : jt * C + (h + 1) * HD],
                                et[:, sl],
                                start=(jt == 0),
                                stop=(jt == JT - 1),
                            )
                            nc.tensor.matmul(
                                dd[:, nt * FT : (nt + 1) * FT],
                                ones,
                                et[:, sl],
                                start=(jt == 0),
                                stop=(jt == JT - 1),
                            )
                    r = scr.tile([P, N], f32, tag="r")
                    ao = aop.tile([P, N], MM_DT, tag=f"ao{h}")
                    for nt in range(NT):
                        sl = slice(nt * FT, (nt + 1) * FT)
                        nc.vector.reciprocal_approx_fast(out=r[:, sl], in_=dd[:, sl])
                        nc.vector.tensor_tensor(
                            ao[:, sl], u[:, sl], r[:, sl], ALU.mult
                        )
                    aos.append(ao)
                return aos

            def proj_out(b, aos):
                xs = xs_all[b]
                pss, os_ = [], []
                for ot in range(CT):
                    ps = pmm.tile([P, N], f32, tag="mm")
                    pss.append(ps)
                    o = op.tile([P, N], f32, tag=f"o{ot}")
                    os_.append(o)
                for nt in range(NT):
                    sl = slice(nt * FT, (nt + 1) * FT)
                    for ot in range(CT):
                        for hh in range(HEADS):
                            nc.tensor.matmul(
                                pss[ot][:, sl],
                                w_ap(WP_, hh)[:, ot * P : (ot + 1) * P],
                                aos[hh][:, sl],
                                start=(hh == 0),
                                stop=(hh == HEADS - 1),
                            )
                    for ot in range(CT):
                        nc.vector.tensor_tensor(
                            os_[ot][:, sl], pss[ot][:, sl], xs[ot][:, sl], ALU.add
                        )
                        nc.sync.dma_start(
                            out_d[b, ot * P : (ot + 1) * P, sl], os_[ot][:, sl]
                        )

            # Interleaved schedule: b1's GN runs during b0's QKV/attention,
            # b1's QKV fills PE while b0's softmax epilogue runs on DVE.
            gn_stats(0)
            xns0 = gn_chain(0)
            # bridge burst: keep PE busy (and HAM warm) while DVE finishes xn
            for _ in range(14):
                wps2 = pacc.tile([P, FT], f32, tag="d")
                nc.tensor.matmul(
                    wps2[:], wtile[:, 0:P], wtile[:], start=True, stop=True
                )
            qkv_b0 = qkv(0, xns0)
            if bpc > 1:
                gn_stats(1)
                xns1 = gn_chain(1)
                aos0 = attn(0, *qkv_b0)
                qkv_b1 = qkv(1, xns1)
                proj_out(0, aos0)
                aos1 = attn(1, *qkv_b1)
                proj_out(1, aos1)
            else:
                aos0 = attn(0, *qkv_b0)
                proj_out(0, aos0)

    nc.compile()
    return nc


def build_const_blob(gn_w, gn_b, wq, wk, wv, wp):
    """Returns (cbw bf16 [P, CB_W], cbg f32 [P, CB_F - CB_W])."""
    import ml_dtypes

    cbw = np.zeros((P, CB_W), np.float32)
    for i, wmat in enumerate((wq, wk, wv, wp)):
        wT = np.asarray(wmat, np.float32).T  # (c_in, c_out)
        for kt in range(CT):
            cbw[:, OFF_W + i * CT * C + kt * C : OFF_W + i * CT * C + (kt + 1) * C] = (
                wT[kt * P : (kt + 1) * P, :]
            )
    cbw[:, OFF_ONES : OFF_ONES + P] = 1.0
    cbg = np.zeros((P, CB_F - CB_W), np.float32)
    gb = OFF_GNWB - CB_W
    cbg[:, gb + 0 : gb + 4 : 2] = np.asarray(gn_w, np.float32).reshape(CT, P).T
    cbg[:, gb + 1 : gb + 4 : 2] = np.asarray(gn_b, np.float32).reshape(CT, P).T
    for ct in range(CT):
        for p in range(P):
            g = (ct * P + p) // GSIZE
            cbg[p, OFF_GMASK - CB_W + ct * G + g] = 1.0 / NG
            cbg[g, OFF_GMT - CB_W + ct * P + p] = 1.0
    cbg[0:G, OFF_EPS - CB_W] = EPS
    return cbw.astype(ml_dtypes.bfloat16), cbg


_NC_CACHE = {}


def kernel(x, gn_w, gn_b, wq, wk, wv, wp):
    x = np.ascontiguousarray(np.asarray(x, dtype=np.float32))
    b, c, h, w = x.shape
    xr = x.reshape(b, c, h * w)
    cbw, cbg = build_const_blob(gn_w, gn_b, wq, wk, wv, wp)

    if "nc" not in _NC_CACHE:
        _NC_CACHE["nc"] = build_bass()
    nc = _NC_CACHE["nc"]

    import ml_dtypes

    xrb = xr.astype(ml_dtypes.bfloat16)
    in_maps = [
        dict(
            x=np.ascontiguousarray(xr[i * BPC : (i + 1) * BPC]),
            xb=np.ascontiguousarray(xrb[i * BPC : (i + 1) * BPC]),
            cbw=cbw,
            cbg=cbg,
        )
        for i in range(N_CORES)
    ]
    res = run_bass_kernel_spmd(nc, in_maps, list(range(N_CORES)))
    out = np.concatenate([res.results[i]["out"] for i in range(N_CORES)], axis=0)
    return out.reshape(b, c, h, w).astype(np.float32)


if __name__ == "__main__":
    rng = np.random.default_rng(0)
    ins = {
        "x": rng.standard_normal((B, C, H, W), dtype=np.float32),
        "gn_w": np.ones((C,), np.float32),
        "gn_b": np.zeros((C,), np.float32),
        "wq": rng.standard_normal((C, C), dtype=np.float32) * C**-0.5,
        "wk": rng.standard_normal((C, C), dtype=np.float32) * C**-0.5,
        "wv": rng.standard_normal((C, C), dtype=np.float32) * C**-0.5,
        "wp": rng.standard_normal((C, C), dtype=np.float32) * C**-0.5,
    }
    out = kernel(**ins)
    print(out.shape, out.dtype)


# revision 45
# speedup vs baseline: 1.0418x; 1.0418x over previous
"""Trainium2 Bass kernel for nn_AttentionBlock (GroupNorm + 2-head attention + proj + residual).

Full inputs: x (16, 256, 32, 32) f32, gn_w/gn_b (256,), wq/wk/wv/wp (256, 256).
Sharding: pure data-parallel over batch — 16 / 8 cores = 2 batch elements per core.
No collectives; outputs concatenated on host.

Per-core dataflow (per batch element, channels-on-partitions):
  x (256, 1024)  -> GroupNorm(4 groups) via free-dim reduces + PE group-mask matmuls
  xn -> q, k (256, 1024) = Wq/Wk @ xn ;  vT (1024, 256) = xn^T @ Wv^T
  per head h (hd = 128): ST_jt (j, i) = k_h[:, jt]^T q_h  (scores transposed)
                         ET = exp(scale * ST)      (ACT, fused scale)
                         U (c, i)  = sum_jt vT_jt_h^T @ ET_jt   (PSUM accum)
                         D (., i)  = sum_jt ones^T @ ET_jt      (softmax denom, replicated)
                         ao_h = U * (1/D)
  out = Wp @ [ao_0; ao_1] + x
All big matmuls run in bf16 (fp32 PSUM accumulation); GroupNorm stats/chain in
fp32. A bf16 copy of x feeds the GN/xn path so the critical input DMA is half
the bytes; fp32 x is only used for the residual. Dummy bf16 warmup matmuls trip
the PE HAM clock gate to 2.4 GHz before real work arrives. Weights and GN
constants arrive as two const-blob DMAs. Emission order interleaves the two
batch elements so b1's GN/QKV hide under b0's attention.
"""

import numpy as np

import concourse.bass as bass
import concourse.tile as tile
from concourse import bacc, mybir
from concourse.bass_utils import run_bass_kernel_spmd

N_CORES = 8
B = 16
BPC = B // N_CORES  # batch elements per core
C = 256
H = W = 32
N = H * W  # 1024 spatial positions
HEADS = 2
HD = C // HEADS  # 128 head dim
G = 4  # groupnorm groups
GSIZE = C // G  # 64 channels per group
EPS = 1e-5
ATT_SCALE = float((C * HEADS) ** -0.5)
P = 128  # partitions
CT = C // P  # channel tiles (2)
FT = 512  # matmul moving-dim tile (one fp32 PSUM bank)
NT = N // FT  # n tiles per matmul row pass (2)
JT = N // P  # j tiles (8)
NG = GSIZE * N  # elements per (batch, group)

# const blob column offsets; region [0, CB_W) is the bf16 weight blob,
# [CB_W, CB_F) the fp32 GN-const blob.
OFF_W = 0  # 4 weights (q,k,v,p), each CT*C = 512 cols
OFF_ONES = 2048  # 128 cols of 1.0
CB_W = 2176
OFF_GNWB = 2176  # per ct: 2 cols
OFF_GMASK = 2184  # per ct: G cols
OFF_GMT = 2192  # per ct: 128 cols (values live in rows 0..G-1)
OFF_EPS = 2448  # one col: EPS in rows 0..G-1
CB_F = 2452

f32 = mybir.dt.float32
f32r = mybir.dt.float32r
bf16 = mybir.dt.bfloat16
MM_DT = bf16  # dtype of all big-matmul operands
N_WARMUP = 16  # PE warmup matmuls (~5us busy) to trip the HAM clock gate early
AF = mybir.ActivationFunctionType
ALU = mybir.AluOpType
AX = mybir.AxisListType


def build_bass(bpc=BPC):
    nc = bacc.Bacc("TRN2", target_bir_lowering=False, debug=False)

    x_d = nc.dram_tensor("x", [bpc, C, N], f32, kind="ExternalInput").ap()
    xb_d = nc.dram_tensor("xb", [bpc, C, N], bf16, kind="ExternalInput").ap()
    cbw_d = nc.dram_tensor("cbw", [P, CB_W], MM_DT, kind="ExternalInput").ap()
    cbg_d = nc.dram_tensor("cbg", [P, CB_F - CB_W], f32, kind="ExternalInput").ap()
    out_d = nc.dram_tensor("out", [bpc, C, N], f32, kind="ExternalOutput").ap()

    with tile.TileContext(nc) as tc:
        with (
            tc.tile_pool(name="consts", bufs=1) as consts,
            tc.tile_pool(name="xp", bufs=2) as xp,
            tc.tile_pool(name="xnp", bufs=2) as xnp,
            tc.tile_pool(name="qk", bufs=2) as qk,
            tc.tile_pool(name="vp", bufs=2) as vp,
            tc.tile_pool(name="etp", bufs=2) as etp,
            tc.tile_pool(name="sm", bufs=2) as sm,
            tc.tile_pool(name="scr", bufs=2) as scr,
            tc.tile_pool(name="aop", bufs=2) as aop,
            tc.tile_pool(name="op", bufs=2) as op,
            tc.tile_pool(name="pmm", bufs=2, space="PSUM") as pmm,
            tc.tile_pool(name="pacc", bufs=1, space="PSUM") as pacc,
        ):
            # ---- PE warmup: dense dummy matmuls (no input deps) so the HAM
            # clock gate reaches K=8/8 before the real matmuls start.
            wtile = consts.tile([P, FT], bf16, tag="warm")
            nc.gpsimd.memset(wtile[:], 0.0)
            wps = pacc.tile([P, FT], f32, tag="u")
            for _ in range(N_WARMUP):
                nc.tensor.matmul(
                    wps[:], wtile[:, 0:P], wtile[:], start=True, stop=True
                )

            # ---- inputs: GN consts + x tiles spread over several DMA queues
            # (sync/gpsimd/vector issue to different HWDGE queues -> parallel)
            cbg = consts.tile([P, CB_F - CB_W], f32, tag="cbg")
            nc.sync.dma_start(cbg[:], cbg_d[:])
            # bf16 copy of x feeds GN stats + xn (half the critical bytes);
            # fp32 x arrives later, used only for the residual add.
            dma_engs = [nc.sync, nc.gpsimd, nc.scalar]
            xb_all = []
            for b in range(bpc):
                xbs = []
                for ct in range(CT):
                    xt = xp.tile([P, N], bf16, tag=f"xb{ct}")
                    eng = dma_engs[(b * CT + ct) % 3]
                    eng.dma_start(xt[:], xb_d[b, ct * P : (ct + 1) * P, :])
                    xbs.append(xt)
                xb_all.append(xbs)
            xs_all = []
            for b in range(bpc):
                xs = []
                for ct in range(CT):
                    xt = xp.tile([P, N], f32, tag=f"x{ct}")
                    eng = dma_engs[(b * CT + ct) % 3]
                    eng.dma_start(xt[:], x_d[b, ct * P : (ct + 1) * P, :])
                    xs.append(xt)
                xs_all.append(xs)

            cbw = consts.tile([P, CB_W], MM_DT, tag="cbw")
            nc.scalar.dma_start(cbw[:], cbw_d[:])

            def w_ap(i, kt):  # (128, C) lhsT/rhs slice of weight i, k-tile kt
                base = OFF_W + i * (CT * C) + kt * C
                return cbw[:, base : base + C]

            def g_ap(off):
                return off - CB_W

            gw = [
                cbg[:, g_ap(OFF_GNWB) + ct * 2 : g_ap(OFF_GNWB) + (ct + 1) * 2]
                for ct in range(CT)
            ]
            gm = [
                cbg[:, g_ap(OFF_GMASK) + ct * G : g_ap(OFF_GMASK) + (ct + 1) * G]
                for ct in range(CT)
            ]
            gmt = [
                cbg[0:G, g_ap(OFF_GMT) + ct * P : g_ap(OFF_GMT) + (ct + 1) * P]
                for ct in range(CT)
            ]
            ones = cbw[:, OFF_ONES : OFF_ONES + P]
            eps_ap = cbg[0:G, g_ap(OFF_EPS) : g_ap(OFF_EPS) + 1]
            WQ, WK, WV, WP_ = 0, 1, 2, 3

            s12_all = {}

            def gn_stats(b):
                xbs = xb_all[b]
                s12s = []
                for ct in range(CT):
                    s12 = sm.tile([P, 2], f32, tag=f"s12_{ct}")
                    nc.vector.reduce_sum(s12[:, 0:1], xbs[ct][:], AX.X)
                    sq = scr.tile([P, N], f32, tag="sq")
                    nc.scalar.activation(
                        sq[:], xbs[ct][:], AF.Square, accum_out=s12[:, 1:2]
                    )
                    s12s.append(s12)
                s12_all[b] = s12s

            def gn_chain(b):
                """gstats matmul -> rstd/mean -> per-channel scale/bias -> xn."""
                s12s = s12_all[b]
                xbs = xb_all[b]
                gstats = pmm.tile([G, 2], f32, tag="mm")
                for ct in range(CT):
                    nc.tensor.matmul(
                        gstats[:],
                        gm[ct],
                        s12s[ct][:],
                        start=(ct == 0),
                        stop=(ct == CT - 1),
                    )
                # gstats = [mean, ex2] (masks pre-scaled by 1/NG on host)
                mrs = sm.tile([G, 2], f32, tag="mrs")  # col0 = rstd, col1 = mean
                nc.vector.tensor_copy(mrs[:, 1:2], gstats[:, 0:1])
                negvar = sm.tile([G, 1], f32, tag="negvar")
                nc.vector.scalar_tensor_tensor(
                    negvar[:],
                    mrs[:, 1:2],
                    mrs[:, 1:2],
                    gstats[:, 1:2],
                    ALU.mult,
                    ALU.subtract,
                )
                std = sm.tile([G, 1], f32, tag="std")
                nc.scalar.activation(
                    std[:], negvar[:], AF.Sqrt, bias=eps_ap, scale=-1.0
                )
                nc.vector.reciprocal(mrs[:, 0:1], std[:])

                sbias = []
                for ct in range(CT):
                    bc = pmm.tile([P, 2], f32, tag="mm")
                    nc.tensor.matmul(bc[:], gmt[ct], mrs[:], start=True, stop=True)
                    scale = sm.tile([P, 1], f32, tag=f"scale{ct}")
                    nc.vector.tensor_tensor(scale[:], bc[:, 0:1], gw[ct][:, 0:1], ALU.mult)
                    nbias = sm.tile([P, 1], f32, tag=f"nbias{ct}")
                    nc.vector.tensor_tensor(nbias[:], bc[:, 1:2], scale[:], ALU.mult)
                    nc.vector.tensor_tensor(
                        nbias[:], gw[ct][:, 1:2], nbias[:], ALU.subtract
                    )
                    sbias.append((scale, nbias))

                xns = []
                for ct in range(CT):
                    xn = xnp.tile([P, N], MM_DT, tag=f"xn{ct}")
                    for nt in range(NT):
                        nc.vector.tensor_scalar(
                            xn[:, nt * FT : (nt + 1) * FT],
                            xbs[ct][:, nt * FT : (nt + 1) * FT],
                            sbias[ct][0][:],
                            sbias[ct][1][:],
                            ALU.mult,
                            ALU.add,
                        )
                    xns.append(xn)
                return xns

            def qkv(b, xns):
                qs, ks = [], []
                for wi, outl, name in ((WQ, qs, "q"), (WK, ks, "k")):
                    for ot in range(CT):
                        ps = pmm.tile([P, N], f32, tag="mm")
                        for nt in range(NT):
                            for kt in range(CT):
                                nc.tensor.matmul(
                                    ps[:, nt * FT : (nt + 1) * FT],
                                    w_ap(wi, kt)[:, ot * P : (ot + 1) * P],
                                    xns[kt][:, nt * FT : (nt + 1) * FT],
                                    start=(kt == 0),
                                    stop=(kt == CT - 1),
                                )
                        t = qk.tile([P, N], MM_DT, tag=f"{name}{ot}")
                        for nt in range(NT):
                            sl = slice(nt * FT, (nt + 1) * FT)
                            if name == "q":
                                nc.scalar.copy(t[:, sl], ps[:, sl])
                            else:
                                nc.vector.tensor_copy(t[:, sl], ps[:, sl])
                        outl.append(t)
                vT = vp.tile([P, JT * C], MM_DT, tag="vt")
                for mt in range(JT):
                    ps = pmm.tile([P, C], f32, tag="mm")
                    for kt in range(CT):
                        nc.tensor.matmul(
                            ps[:],
                            xns[kt][:, mt * P : (mt + 1) * P],
                            w_ap(WV, kt),
                            start=(kt == 0),
                            stop=(kt == CT - 1),
                        )
                    nc.vector.tensor_copy(vT[:, mt * C : (mt + 1) * C], ps[:])
                return qs, ks, vT

            def attn(b, qs, ks, vT, filler=None):
                aos = []
                for h in range(HEADS):
                    qh, kh = qs[h], ks[h]
                    et = etp.tile([P, JT * N], MM_DT, tag="et")
                    for jt in range(JT):
                        st = pmm.tile([P, N], f32, tag="mm")
                        for nt in range(NT):
                            nc.tensor.matmul(
                                st[:, nt * FT : (nt + 1) * FT],
                                kh[:, jt * P : (jt + 1) * P],
                                qh[:, nt * FT : (nt + 1) * FT],
                                start=True,
                                stop=True,
                            )
                        nc.scalar.activation(
                            et[:, jt * N : (jt + 1) * N],
                            st[:],
                            AF.Exp,
                            scale=ATT_SCALE,
                        )
                    u = pacc.tile([P, N], f32, tag="u")
                    dd = pacc.tile([P, N], f32, tag="d")
                    for jt in range(JT):
                        if filler is not None and h == HEADS - 1 and jt == JT - 2:
                            filler()
                            filler = None
                        for nt in range(NT):
                            sl = slice(jt * N + nt * FT, jt * N + (nt + 1) * FT)
                            nc.tensor.matmul(
                                u[:, nt * FT : (nt + 1) * FT],
                                vT[:, jt * C + h * HD : jt * C + (h + 1) * HD],
                                et[:, sl],
                                start=(jt == 0),
                                stop=(jt == JT - 1),
                            )
                            nc.tensor.matmul(
                                dd[:, nt * FT : (nt + 1) * FT],
                                ones,
                                et[:, sl],
                                start=(jt == 0),
                                stop=(jt == JT - 1),
                            )
                    r = scr.tile([P, N], f32, tag="r")
                    ao = aop.tile([P, N], MM_DT, tag=f"ao{h}")
                    for nt in range(NT):
                        sl = slice(nt * FT, (nt + 1) * FT)
                        nc.vector.reciprocal_approx_fast(out=r[:, sl], in_=dd[:, sl])
                        nc.vector.tensor_tensor(
                            ao[:, sl], u[:, sl], r[:, sl], ALU.mult
                        )
                    aos.append(ao)
                return aos

            def proj_out(b, aos):
                xs = xs_all[b]
                pss, os_ = [], []
                for ot in range(CT):
                    ps = pmm.tile([P, N], f32, tag="mm")
                    pss.append(ps)
                    o = op.tile([P, N], f32, tag=f"o{ot}")
                    os_.append(o)
                for nt in range(NT):
                    sl = slice(nt * FT, (nt + 1) * FT)
                    for ot in range(CT):
                        for hh in range(HEADS):
                            nc.tensor.matmul(
                                pss[ot][:, sl],
                                w_ap(WP_, hh)[:, ot * P : (ot + 1) * P],
                                aos[hh][:, sl],
                                start=(hh == 0),
                                stop=(hh == HEADS - 1),
                            )
                    for ot in range(CT):
                        nc.vector.tensor_tensor(
                            os_[ot][:, sl], pss[ot][:, sl], xs[ot][:, sl], ALU.add
                        )
                        nc.sync.dma_start(
                            out_d[b, ot * P : (ot + 1) * P, sl], os_[ot][:, sl]
                        )

            # Interleaved schedule: b1's GN runs during b0's QKV/attention,
            # b1's QKV fills PE while b0's softmax epilogue runs on DVE.
            gn_stats(0)
            xns0 = gn_chain(0)
            # bridge burst: keep PE busy (and HAM warm) while DVE finishes xn
            wps2 = pacc.tile([P, FT], f32, tag="d")
            for _ in range(16):
                nc.tensor.matmul(
                    wps2[:], wtile[:, 0:P], wtile[:], start=True, stop=True
                )
            qkv_b0 = qkv(0, xns0)
            if bpc > 1:
                gn_stats(1)
                xns1 = gn_chain(1)
                aos0 = attn(0, *qkv_b0)
                qkv_b1 = qkv(1, xns1)
                proj_out(0, aos0)
                aos1 = attn(1, *qkv_b1)
                proj_out(1, aos1)
            else:
                aos0 = attn(0, *qkv_b0)
                proj_out(0, aos0)

    nc.compile()
    return nc


def build_const_blob(gn_w, gn_b, wq, wk, wv, wp):
    """Returns (cbw bf16 [P, CB_W], cbg f32 [P, CB_F - CB_W])."""
    import ml_dtypes

    cbw = np.zeros((P, CB_W), np.float32)
    for i, wmat in enumerate((wq, wk, wv, wp)):
        wT = np.asarray(wmat, np.float32).T  # (c_in, c_out)
        for kt in range(CT):
            cbw[:, OFF_W + i * CT * C + kt * C : OFF_W + i * CT * C + (kt + 1) * C] = (
                wT[kt * P : (kt + 1) * P, :]
            )
    cbw[:, OFF_ONES : OFF_ONES + P] = 1.0
    cbg = np.zeros((P, CB_F - CB_W), np.float32)
    gb = OFF_GNWB - CB_W
    cbg[:, gb + 0 : gb + 4 : 2] = np.asarray(gn_w, np.float32).reshape(CT, P).T
    cbg[:, gb + 1 : gb + 4 : 2] = np.asarray(gn_b, np.float32).reshape(CT, P).T
    for ct in range(CT):
        for p in range(P):
            g = (ct * P + p) // GSIZE
            cbg[p, OFF_GMASK - CB_W + ct * G + g] = 1.0 / NG
            cbg[g, OFF_GMT - CB_W + ct * P + p] = 1.0
    cbg[0:G, OFF_EPS - CB_W] = EPS
    return cbw.astype(ml_dtypes.bfloat16), cbg


_NC_CACHE = {}


def kernel(x, gn_w, gn_b, wq, wk, wv, wp):
    x = np.ascontiguousarray(np.asarray(x, dtype=np.float32))
    b, c, h, w = x.shape
    xr = x.reshape(b, c, h * w)
    cbw, cbg = build_const_blob(gn_w, gn_b, wq, wk, wv, wp)

    if "nc" not in _NC_CACHE:
        _NC_CACHE["nc"] = build_bass()
    nc = _NC_CACHE["nc"]

    import ml_dtypes

    xrb = xr.astype(ml_dtypes.bfloat16)
    in_maps = [
        dict(
            x=np.ascontiguousarray(xr[i * BPC : (i + 1) * BPC]),
            xb=np.ascontiguousarray(xrb[i * BPC : (i + 1) * BPC]),
            cbw=cbw,
            cbg=cbg,
        )
        for i in range(N_CORES)
    ]
    res = run_bass_kernel_spmd(nc, in_maps, list(range(N_CORES)))
    out = np.concatenate([res.results[i]["out"] for i in range(N_CORES)], axis=0)
    return out.reshape(b, c, h, w).astype(np.float32)


if __name__ == "__main__":
    rng = np.random.default_rng(0)
    ins = {
        "x": rng.standard_normal((B, C, H, W), dtype=np.float32),
        "gn_w": np.ones((C,), np.float32),
        "gn_b": np.zeros((C,), np.float32),
        "wq": rng.standard_normal((C, C), dtype=np.float32) * C**-0.5,
        "wk": rng.standard_normal((C, C), dtype=np.float32) * C**-0.5,
        "wv": rng.standard_normal((C, C), dtype=np.float32) * C**-0.5,
        "wp": rng.standard_normal((C, C), dtype=np.float32) * C**-0.5,
    }
    out = kernel(**ins)
    print(out.shape, out.dtype)
